# revision 1
# baseline (speedup 1.0000x reference)
"""Trainium2 Bass kernel for nn_BaseTBLoRa (moe_routing).

out[b,s,:] = x[b,s,:] @ W_base.T + b_base + 2.0 * ((x[b,s,:] @ A_w[e_b].T) @ B_w[e_b].T)
with e_b = segment[b].  B=8, S=2048, D=1024, Do=1024, R=16, E=8.

Sharding: data-parallel over batch — core b handles batch b (B == n_cores).
The expert routing (segment -> A/B weights) is resolved on the host; each
core receives only its selected LoRA weights. The host pre-transposes all
operands so the contraction dim lands on SBUF partitions, and LORA_SCALE is
folded into B.

Device math per core (matmuls in bf16, fp32 PSUM accumulation; float32r was
tried first — numerically fine but ~180x below its cost-model rate on real HW):

  for each 512-wide s macro-chunk (4 of them):
    ps_h[r, s512]   = sum_dt AT[dt][:, :16].T @ xT[dt, s512]     (8 MMs, N=512)
    h_pad           = [bf16(ps_h); ones-row; zeros]              (128 rows)
    for each 128-row s-tile (4 per chunk):
      ps_y[s128, :] = sum_dt xT[dt, s128].T @ WT[dt]             (16 MMs, N=512)
      ps_y[s128, :] += h_pad[:, s128].T @ BT_pad                 (2 MMs, N=512)
      out <- DVE copy of ps_y, DMA to DRAM

Key measured-on-HW facts driving the shape:
  - bf16 MM N=512: ~240-250ns (LDW overlaps); K=16 MM: ~450ns -> the LoRA
    matmul contraction is zero-padded to K=128 (stream time is N-cycles
    regardless of K, so the pad rows are free).
  - The ones-row in h_pad times a b_base row in BT_pad adds the bias inside
    the same matmul, removing a separate DVE bias-add.
  - Engine memsets need 32-aligned base partitions, so the constant pad rows
    (ones/zeros) are DMA'd from a small host-supplied tensor instead.

Steady-state body time measured by For_i dilation: ~85-90us/core (cool chip),
vs a ~78us PE-stream floor (4.43 GFLOP/core at 78.6 TF/s bf16 + overheads).
"""

import ml_dtypes
import numpy as np

import concourse.tile as tile
from concourse import bacc, mybir
from concourse.bass_utils import run_bass_kernel_spmd

LORA_SCALE = 32.0 / 16.0

B, S, D, DO, R = 8, 2048, 1024, 1024, 16
NDT = D // 128   # 8 contraction tiles
NST = S // 128   # 16 s-tiles
NSC = 4          # s macro-chunks
SC = S // NSC    # 512 s per macro-chunk
SUB = SC // 128  # 4 s-tiles per macro-chunk
N_CORES = 8

F32 = mybir.dt.float32
BF16 = mybir.dt.bfloat16

last_in_maps = None
last_results = None


def _build(loop_n=0):
    """loop_n > 0 wraps the body in a dynamic For_i (used only for dilation
    timing); the graded path uses loop_n=0 (straight-line program)."""
    import contextlib

    nc = bacc.Bacc("TRN2", target_bir_lowering=False, debug=False)

    x_d = nc.dram_tensor("x5", [NSC, 128, NDT, SC], BF16, kind="ExternalInput")
    w_d = nc.dram_tensor("wt", [NDT, 128, DO], BF16, kind="ExternalInput")
    a_d = nc.dram_tensor("at", [128, NDT, R], BF16, kind="ExternalInput")
    bt_d = nc.dram_tensor("bt", [128, DO], BF16, kind="ExternalInput")
    hpc_d = nc.dram_tensor("hpc", [128 - R, SC], BF16, kind="ExternalInput")
    out_d = nc.dram_tensor("out", [NST, 128, DO], F32, kind="ExternalOutput")

    with tile.TileContext(nc) as tc:
        with (
            tc.tile_pool(name="wpool", bufs=1) as wpool,
            tc.tile_pool(name="cpool", bufs=1) as cpool,
            tc.tile_pool(name="xpool", bufs=3) as xpool,
            tc.tile_pool(name="hpool", bufs=2) as hpool,
            tc.tile_pool(name="opool", bufs=4) as opool,
            tc.tile_pool(name="psy", bufs=2, space="PSUM") as psy,
            tc.tile_pool(name="psh", bufs=2, space="PSUM") as psh,
        ):
            loop_cm = tc.For_i(0, loop_n, 1) if loop_n else contextlib.nullcontext()

            def preload():
                a_t = cpool.tile([128, NDT, R], BF16)
                nc.sync.dma_start(a_t[:], a_d[:])
                bt_t = cpool.tile([128, DO], BF16)
                nc.sync.dma_start(bt_t[:], bt_d[:])
                w_t = wpool.tile([128, NDT, DO], BF16)
                return a_t, bt_t, w_t

            # A For_i body may not touch tiles allocated outside the loop, so
            # in timing mode the preload moves inside (slightly conservative).
            if not loop_n:
                a_t, bt_t, w_t = preload()
            with loop_cm:
                if loop_n:
                    a_t, bt_t, w_t = preload()
                for sc in range(NSC):
                    x_t = xpool.tile([128, NDT, SC], BF16)
                    for dt in range(NDT):
                        nc.sync.dma_start(x_t[:, dt, :], x_d[sc, :, dt, :])
                        if sc == 0:
                            # interleave the W preload with the first x chunk
                            nc.sync.dma_start(w_t[:, dt, :], w_d[dt])

                    # hT for the whole macro-chunk first (N=512 streams); the
                    # copy to SBUF overlaps the base matmuls below.
                    ps_h = psh.tile([R, SC], F32)
                    for dt in range(NDT):
                        nc.tensor.matmul(
                            ps_h[:], a_t[:, dt, :], x_t[:, dt, :],
                            start=(dt == 0), stop=(dt == NDT - 1),
                        )
                    h_t = hpool.tile([128, SC], BF16)
                    nc.sync.dma_start(h_t[R:128, :], hpc_d[:])
                    nc.vector.tensor_copy(h_t[0:R, :], ps_h[:])

                    for sub in range(SUB):
                        st = sc * SUB + sub
                        ps_y = psy.tile([128, DO], F32)
                        for dt in range(NDT):
                            xt = x_t[:, dt, sub * 128:(sub + 1) * 128]
                            first = dt == 0
                            nc.tensor.matmul(
                                ps_y[:, 0:512], xt, w_t[:, dt, 0:512],
                                start=first, stop=False,
                            )
                            nc.tensor.matmul(
                                ps_y[:, 512:1024], xt, w_t[:, dt, 512:1024],
                                start=first, stop=False,
                            )
                        hs = h_t[:, sub * 128:(sub + 1) * 128]
                        nc.tensor.matmul(
                            ps_y[:, 0:512], hs, bt_t[:, 0:512],
                            start=False, stop=True,
                        )
                        nc.tensor.matmul(
                            ps_y[:, 512:1024], hs, bt_t[:, 512:1024],
                            start=False, stop=True,
                        )

                        o_t = opool.tile([128, DO], F32)
                        nc.vector.tensor_copy(o_t[:], ps_y[:])
                        nc.sync.dma_start(out_d[st], o_t[:])

    nc.compile()
    return nc


def _prep_core_inputs(x_b, e, W_base, b_base, A_w, B_w, wt5):
    xT = x_b.T.reshape(NDT, 128, NSC, SC).transpose(2, 1, 0, 3).astype(
        ml_dtypes.bfloat16
    )
    at = A_w[e].T.reshape(NDT, 128, R).transpose(1, 0, 2).astype(ml_dtypes.bfloat16)
    bt = np.zeros((128, DO), dtype=ml_dtypes.bfloat16)
    bt[0:R] = (LORA_SCALE * B_w[e].T).astype(ml_dtypes.bfloat16)
    bt[R] = b_base.astype(ml_dtypes.bfloat16)  # pairs with the ones-row in h_pad
    hpc = np.zeros((128 - R, SC), dtype=ml_dtypes.bfloat16)
    hpc[0] = 1.0  # h_pad row R: multiplies the bias row of bt
    return {"x5": xT, "wt": wt5, "at": at, "bt": bt, "hpc": hpc}


def kernel(x, segment, W_base, b_base, A_w, B_w, _sim=False):
    global last_in_maps, last_results

    x = np.asarray(x, dtype=np.float32)
    W_base = np.asarray(W_base, dtype=np.float32)
    b_base = np.asarray(b_base, dtype=np.float32)
    A_w = np.asarray(A_w, dtype=np.float32)
    B_w = np.asarray(B_w, dtype=np.float32)
    seg = np.asarray(segment).astype(np.int64)

    wt5 = np.ascontiguousarray(W_base.T).reshape(NDT, 128, DO).astype(
        ml_dtypes.bfloat16
    )

    in_maps = [
        _prep_core_inputs(x[b], int(seg[b]), W_base, b_base, A_w, B_w, wt5)
        for b in range(B)
    ]
    last_in_maps = in_maps

    nc = _build()

    if _sim:
        from concourse.bass_interp import CoreSim

        outs = []
        for b in range(B):
            sim = CoreSim(nc)
            for name, arr in in_maps[b].items():
                sim.tensor(name)[:] = arr
            sim.simulate()
            outs.append(np.array(sim.tensor("out")).reshape(S, DO))
        return np.stack(outs)

    res = run_bass_kernel_spmd(nc, in_maps, list(range(N_CORES)))
    last_results = res
    return np.stack([res.results[c]["out"].reshape(S, DO) for c in range(N_CORES)])

